# revision 5
# baseline (speedup 1.0000x reference)
"""Trainium2 Bass kernel for nn_AlignmentVAE (retrieval_knn).

Strategy
--------
reference() needs argmin_j d2(i,j) for every src row i (indices1) and
argmin_i d2(i,j) for every dst row j (indices2), then a cheap O(N)
mean-of-|diff| on the gathered neighbors.

We run TWO row-argmin problems (so no cross-partition/col reductions and
no collectives):
  dir 1: rows = pointsI (sharded 2048/core), cols = all pointsJ
  dir 2: rows = pointsJ (sharded 2048/core), cols = all pointsI

d2 is produced by ONE fp16 matmul per tile using an augmented-K trick:
  -d2(i,j) = 2 x_i x_j + 2 y_i y_j - |p_i|^2 - |p_j|^2
Each fp32 coordinate is split hi/lo into two fp16 values (x = xh + xl),
which makes every product exact in fp32 PSUM and keeps the effective
precision at ~22 mantissa bits; K=12 slots give the full expansion.
fp16 matmul streams at 1 cycle/row (fp32 would be 4x slower).

Per 128-row stripe: 32 matmuls -> PSUM, ACT copies PSUM->SBUF (fp16),
then DVE InstMax (top-8) + InstMaxIndex give the row max of -d2 and its
FIRST-occurrence index == jnp.argmin tie semantics. Host gathers the
2048 indices per core per direction and finishes the O(N) scalar.
"""

import numpy as np
from contextlib import ExitStack

import concourse.bass as bass
import concourse.bacc as bacc
import concourse.mybir as mybir
import concourse.tile as tile
from concourse.bass_utils import run_bass_kernel_spmd

N = 16384
M = 16384
NCORES = 8
RPC = N // NCORES          # rows per core per direction (2048)
K = 12                     # augmented contraction slots
STRIPES = RPC // 128       # 16
CHUNK = 2048               # PSUM tile free size (4 banks)
NCHUNK = M // CHUNK        # 8
F16 = mybir.dt.float16
F32 = mybir.dt.float32
U32 = mybir.dt.uint32

_prog_cache = {}


def _build_program():
    nc = bacc.Bacc("TRN2", target_bir_lowering=False, debug=False)

    u1 = nc.dram_tensor("u1", [K, RPC], F16, kind="ExternalInput").ap()
    v1 = nc.dram_tensor("v1", [K, M], F16, kind="ExternalInput").ap()
    u2 = nc.dram_tensor("u2", [K, RPC], F16, kind="ExternalInput").ap()
    v2 = nc.dram_tensor("v2", [K, N], F16, kind="ExternalInput").ap()
    o1 = nc.dram_tensor("o1", [STRIPES, 128], U32, kind="ExternalOutput").ap()
    o2 = nc.dram_tensor("o2", [STRIPES, 128], U32, kind="ExternalOutput").ap()

    with tile.TileContext(nc) as tc, ExitStack() as ctx:
        const = ctx.enter_context(tc.tile_pool(name="const", bufs=1))
        psum = ctx.enter_context(tc.tile_pool(name="psum", bufs=2, space="PSUM"))
        stripes = ctx.enter_context(tc.tile_pool(name="stripes", bufs=2))
        small = ctx.enter_context(tc.tile_pool(name="small", bufs=4))

        u1sb = const.tile([K, RPC], F16, tag="u1")
        nc.sync.dma_start(u1sb[:], u1[:])
        v1sb = const.tile([K, M], F16, tag="v1")
        nc.sync.dma_start(v1sb[:], v1[:])
        u2sb = const.tile([K, RPC], F16, tag="u2")
        nc.sync.dma_start(u2sb[:], u2[:])
        v2sb = const.tile([K, M], F16, tag="v2")
        nc.sync.dma_start(v2sb[:], v2[:])
        tc.strict_bb_all_engine_barrier()

        for usb, vsb, O in ((u1sb, v1sb, o1), (u2sb, v2sb, o2)):
            for s in range(STRIPES):
                stripe = stripes.tile([128, M], F16, tag="stripe")
                lhs = usb[:, s * 128:(s + 1) * 128]
                for c in range(NCHUNK):
                    pt = psum.tile([128, CHUNK], F32, tag="pt")
                    for q in range(CHUNK // 512):
                        off = c * CHUNK + q * 512
                        nc.tensor.matmul(
                            pt[:, q * 512:(q + 1) * 512],
                            lhs,
                            vsb[:, off:off + 512],
                            start=True, stop=True,
                        )
                    nc.scalar.activation(
                        stripe[:, c * CHUNK:(c + 1) * CHUNK], pt[:],
                        mybir.ActivationFunctionType.Copy,
                    )
                top8 = small.tile([128, 8], F16, tag="top8")
                nc.vector.max(top8[:], stripe[:])
                idx8 = small.tile([128, 8], U32, tag="idx8")
                nc.vector.max_index(idx8[:], top8[:], stripe[:])
                nc.sync.dma_start(O[s], idx8[:, 0:1])
    nc.finalize()
    return nc


def _split16(x):
    """fp32 -> (hi, lo) fp16 pair with x ~= hi + lo."""
    h = x.astype(np.float16)
    l = (x - h.astype(np.float32)).astype(np.float16)
    return h, l


def _aug(points):
    """points [n,2] fp32 -> (U [12,n] fp16, V [12,n] fp16).

    sum_k U[k,i] * V[k,j] == 2 x_i x_j + 2 y_i y_j - |p_i|^2 - |p_j|^2
    """
    x = np.ascontiguousarray(points[:, 0]).astype(np.float32)
    y = np.ascontiguousarray(points[:, 1]).astype(np.float32)
    xh, xl = _split16(x)
    yh, yl = _split16(y)
    sq = x * x + y * y
    sh, sl = _split16(sq)
    two = np.float32(2.0)
    xh2 = (xh.astype(np.float32) * two).astype(np.float16)
    xl2 = (xl.astype(np.float32) * two).astype(np.float16)
    yh2 = (yh.astype(np.float32) * two).astype(np.float16)
    yl2 = (yl.astype(np.float32) * two).astype(np.float16)
    ones = np.ones_like(xh)
    U = np.stack([xh2, xh2, xl2, xl2, yh2, yh2, yl2, yl2, -sh, -sl, ones, ones])
    V = np.stack([xh, xl, xh, xl, yh, yl, yh, yl, ones, ones, -sh, -sl])
    return np.ascontiguousarray(U), np.ascontiguousarray(V)


def _prep_inputs(pI, pJ):
    UI, VI = _aug(pI)
    UJ, VJ = _aug(pJ)
    in_maps = []
    for c in range(NCORES):
        sl = slice(c * RPC, (c + 1) * RPC)
        in_maps.append({
            "u1": np.ascontiguousarray(UI[:, sl]),
            "v1": VJ,
            "u2": np.ascontiguousarray(UJ[:, sl]),
            "v2": VI,
        })
    return in_maps


def kernel(pointsI, pointsJ):
    pI = np.asarray(pointsI, dtype=np.float32)
    pJ = np.asarray(pointsJ, dtype=np.float32)

    if "nc" not in _prog_cache:
        _prog_cache["nc"] = _build_program()
    nc = _prog_cache["nc"]

    in_maps = _prep_inputs(pI, pJ)
    res = run_bass_kernel_spmd(nc, in_maps, list(range(NCORES))).results

    idx1 = np.concatenate([res[c]["o1"].reshape(-1) for c in range(NCORES)])
    idx2 = np.concatenate([res[c]["o2"].reshape(-1) for c in range(NCORES)])
    idx1 = idx1.astype(np.int64)
    idx2 = idx2.astype(np.int64)

    err_i = np.mean(np.abs(pI.astype(np.float64) - pJ[idx1].astype(np.float64)))
    err_j = np.mean(np.abs(pJ.astype(np.float64) - pI[idx2].astype(np.float64)))
    out = err_i / N + err_j / M
    return np.array(out, dtype=np.float32)


# revision 6
# speedup vs baseline: 1.1374x; 1.1374x over previous
"""Trainium2 Bass kernel for nn_AlignmentVAE (retrieval_knn, N=M=16384, 2-D).

reference() = argmin_j d2(i,j) per src row (indices1), argmin_i per dst
row (indices2), then an O(N) mean |pI - pJ[idx]| scalar. We solve TWO
row-argmin problems (dir 1: rows=pointsI, cols=pointsJ; dir 2 swapped),
sharding rows 2048/core over 8 NeuronCores — no collectives needed.

Device algorithm (per core, per direction):
- d2 tiles come from ONE fp16 matmul per stripe via an augmented-K trick:
    -d2(i,j) = 2 x_i x_j + 2 y_i y_j - |p_i|^2 - |p_j|^2
  with every fp32 input split hi/lo into two fp16 values (K=12 slots),
  making each product exact in fp32 PSUM (~22-bit effective precision;
  fp16 streams the PE at 1 cycle/row vs 4 for fp32).
- Host sorts both point sets by x. Each 128-row stripe only scans a
  W=512 column window in rank space (banded). Exactness: the host
  computes an upper bound UB_i on each row's NN distance (min over a
  256-point sample + 32 rank-neighbors); rows whose [x-UB, x+UB] rank
  span exceeds their stripe window go to a 128-row overflow block
  computed at full width, column-sharded across the 8 cores and combined
  on the host. Result == full-width brute force argmin.
- Per stripe: matmul -> PSUM [128,512]; ACT copies PSUM->SBUF fp16; DVE
  InstMax (top-8) + InstMaxIndex give the row max of -d2 and its
  FIRST-occurrence index (== jnp.argmin tie semantics). Indices land in
  an SBUF staging tile; ONE output DMA per direction.
- All per-direction inputs are packed into one DRAM blob (chunked DMA,
  single semaphore per consumer — walrus limits waits per instruction).

Host finishes with the O(N) gather + mean (the unshard step).
"""

import numpy as np
from contextlib import ExitStack

import concourse.bass as bass
import concourse.bacc as bacc
import concourse.mybir as mybir
import concourse.tile as tile
from concourse.bass_utils import run_bass_kernel_spmd

N = 16384
M = 16384
NCORES = 8
RPC = N // NCORES          # 2048 rows per core per direction
K = 12
STRIPES = RPC // 128       # 16
W = 512                    # banded column window
OVF = 128                  # overflow rows per direction (padded)
OVB = OVF // 128           # overflow row-blocks
OVW = M // NCORES          # overflow column shard width per core (2048)
SAMPLE = 256
LOCAL = 32
F16 = mybir.dt.float16
F32 = mybir.dt.float32
U32 = mybir.dt.uint32

# blob layout: 16 interleaved stripe groups [u_s(128) | w_s(W)] then [uo | vo]
SW = 128 + W                 # columns per stripe group
SPC = 4                      # stripe groups per DMA chunk
NCHUNK_DMA = STRIPES // SPC + 1   # 4 stripe chunks + 1 ovf chunk
OFF_UO = STRIPES * SW
OFF_VO = OFF_UO + OVF
BLOB = OFF_VO + OVW

_prog_cache = {}


def _build_program():
    nc = bacc.Bacc("TRN2", target_bir_lowering=False, debug=False)

    ins = {}
    outs = {}
    for d in (1, 2):
        ins[d] = nc.dram_tensor(f"b{d}", [K, BLOB], F16, kind="ExternalInput").ap()
        outs[f"qi{d}"] = nc.dram_tensor(f"qi{d}", [128, 8 * (STRIPES + OVB)], U32,
                                        kind="ExternalOutput").ap()
        outs[f"qf{d}"] = nc.dram_tensor(f"qf{d}", [128, 8 * OVB], F16,
                                        kind="ExternalOutput").ap()

    with tile.TileContext(nc) as tc, ExitStack() as ctx:
        const = ctx.enter_context(tc.tile_pool(name="const", bufs=1))
        psum = ctx.enter_context(tc.tile_pool(name="psum", bufs=4, space="PSUM"))
        wins = ctx.enter_context(tc.tile_pool(name="wins", bufs=10))
        winso = ctx.enter_context(tc.tile_pool(name="winso", bufs=2))
        small = ctx.enter_context(tc.tile_pool(name="small", bufs=8))
        stage = ctx.enter_context(tc.tile_pool(name="stage", bufs=1))

        # chunked blob load (tiny first chunk so compute starts early)
        blob = {}
        bounds = [0, 1, 4, 8, 12, 16]
        for d in (1, 2):
            t = const.tile([K, BLOB], F16, tag=f"blob{d}")
            for i in range(len(bounds) - 1):
                lo, hi = bounds[i] * SW, bounds[i + 1] * SW
                nc.sync.dma_start(t[:, lo:hi], ins[d][:, lo:hi])
                if bounds[i + 1] == 4:
                    nc.sync.dma_start(t[:, OFF_UO:BLOB], ins[d][:, OFF_UO:BLOB])
            blob[d] = t

        st_i, st_f = {}, {}
        for d in (1, 2):
            st_i[d] = stage.tile([128, 8 * (STRIPES + OVB)], U32, tag=f"sti{d}", name=f"sti{d}")
            st_f[d] = stage.tile([128, 8 * OVB], F16, tag=f"stf{d}", name=f"stf{d}")

        G = 4  # stripes per phase group

        def banded_group(g):
            pts, wns = {}, {}
            for d in (1, 2):
                b = blob[d]
                for i in range(G):
                    s = g * G + i
                    pt = psum.tile([128, W], F32, tag=f"pt{d}")
                    nc.tensor.matmul(pt[:], b[:, s * SW:s * SW + 128],
                                     b[:, s * SW + 128:(s + 1) * SW],
                                     start=True, stop=True)
                    pts[d, i] = pt
            for d in (1, 2):
                for i in range(G):
                    win = wins.tile([128, W], F16, tag=f"win{d}")
                    nc.scalar.activation(win[:], pts[d, i][:],
                                         mybir.ActivationFunctionType.Copy)
                    wns[d, i] = win
            t8 = {}
            for d in (1, 2):
                top8 = small.tile([128, 8 * G], F16, tag=f"top8{d}", name=f"top8{d}")
                for i in range(G):
                    nc.vector.max(top8[:, 8 * i:8 * i + 8], wns[d, i][:])
                t8[d] = top8
            for d in (1, 2):
                for i in range(G):
                    s = g * G + i
                    nc.vector.max_index(st_i[d][:, 8 * s:8 * s + 8],
                                        t8[d][:, 8 * i:8 * i + 8], wns[d, i][:])

        def ovf_block(d, ob):
            b = blob[d]
            wino = winso.tile([128, OVW], F16, tag="wino")
            for q in range(OVW // W):
                pto = psum.tile([128, W], F32, tag=f"pt{d}")
                nc.tensor.matmul(pto[:], b[:, OFF_UO + ob * 128:OFF_UO + (ob + 1) * 128],
                                 b[:, OFF_VO + q * W:OFF_VO + (q + 1) * W],
                                 start=True, stop=True)
                nc.scalar.activation(wino[:, q * W:(q + 1) * W], pto[:],
                                     mybir.ActivationFunctionType.Copy)
            nc.vector.max(st_f[d][:, 8 * ob:8 * ob + 8], wino[:])
            nc.vector.max_index(st_i[d][:, 8 * (STRIPES + ob):8 * (STRIPES + ob) + 8],
                                st_f[d][:, 8 * ob:8 * ob + 8], wino[:])

        for g in range(STRIPES // G):
            banded_group(g)
            if g == 0:
                for ob in range(OVB):
                    ovf_block(1, ob)
                    ovf_block(2, ob)
        for d in (1, 2):
            nc.sync.dma_start(outs[f"qi{d}"][:], st_i[d][:])
            nc.sync.dma_start(outs[f"qf{d}"][:], st_f[d][:])
    nc.finalize()
    return nc


def _split16(x):
    h = x.astype(np.float16)
    l = (x - h.astype(np.float32)).astype(np.float16)
    return h, l


def _aug(points):
    x = np.ascontiguousarray(points[:, 0]).astype(np.float32)
    y = np.ascontiguousarray(points[:, 1]).astype(np.float32)
    xh, xl = _split16(x)
    yh, yl = _split16(y)
    sq = x * x + y * y
    sh, sl = _split16(sq)
    two = np.float32(2.0)
    d = lambda a: (a.astype(np.float32) * two).astype(np.float16)
    ones = np.ones_like(xh)
    U = np.stack([d(xh), d(xh), d(xl), d(xl), d(yh), d(yh), d(yl), d(yl),
                  -sh, -sl, ones, ones])
    V = np.stack([xh, xl, xh, xl, yh, yl, yh, yl, ones, ones, -sh, -sl])
    return np.ascontiguousarray(U), np.ascontiguousarray(V)


def _plan_direction(rows_pts, cols_pts):
    """Sort, bound, place windows. Returns everything the host needs to
    build inputs and decode outputs for one direction."""
    pr = np.argsort(rows_pts[:, 0], kind="stable")
    pc = np.argsort(cols_pts[:, 0], kind="stable")
    R = rows_pts[pr].astype(np.float32)
    C = cols_pts[pc].astype(np.float32)
    m = C.shape[0]
    xc = C[:, 0].astype(np.float64)

    samp = C[:: m // SAMPLE]
    ub2 = ((R[:, None, :] - samp[None, :, :]) ** 2).sum(-1).min(1)
    rk = np.searchsorted(xc, R[:, 0].astype(np.float64))
    offs = np.arange(-LOCAL // 2, LOCAL // 2)
    nb = np.clip(rk[:, None] + offs[None, :], 0, m - 1)
    ub2 = np.minimum(ub2, ((R[:, None, :] - C[nb]) ** 2).sum(-1).min(1))
    UB = np.sqrt(ub2.astype(np.float64)) * (1 + 1e-6) + 1e-7
    lo_need = np.searchsorted(xc, R[:, 0].astype(np.float64) - UB, side="left")
    hi_need = np.searchsorted(xc, R[:, 0].astype(np.float64) + UB, side="right")

    n = R.shape[0]
    los = np.zeros(n // 128, np.int64)
    ovf_rows = []
    for s in range(n // 128):
        rows = slice(s * 128, (s + 1) * 128)
        ln, hn = lo_need[rows], hi_need[rows]
        cand = [int(np.clip(int(np.median((ln + hn) // 2)) - W // 2, 0, m - W)),
                int(np.clip(ln.min(), 0, m - W))]
        bads = [((ln < lo) | (hn > lo + W)).sum() for lo in cand]
        lo = cand[int(np.argmin(bads))]
        los[s] = lo
        bad = (ln < lo) | (hn > lo + W)
        ovf_rows.extend((s * 128 + np.nonzero(bad)[0]).tolist())

    UR, _ = _aug(R)
    _, VC = _aug(C)
    return dict(pr=pr, pc=pc, UR=UR, VC=VC, los=los,
                ovf_rows=np.array(ovf_rows, np.int64), n=n, m=m)


def _prep(pI, pJ):
    plans = {1: _plan_direction(pI, pJ), 2: _plan_direction(pJ, pI)}
    in_maps = [dict() for _ in range(NCORES)]
    for d, pl in plans.items():
        UR, VC, los = pl["UR"], pl["VC"], pl["los"]
        ovf = pl["ovf_rows"]
        if len(ovf) > OVF:
            pl["ovf_extra"] = ovf[OVF:]
            ovf = ovf[:OVF]
        else:
            pl["ovf_extra"] = np.array([], np.int64)
        ovf_pad = np.concatenate([ovf, np.zeros(OVF - len(ovf), np.int64)])
        pl["ovf_used"] = ovf
        uo = UR[:, ovf_pad]
        for c in range(NCORES):
            b = np.empty((K, BLOB), np.float16)
            for s in range(STRIPES):
                lo = los[c * STRIPES + s]
                g = c * RPC + s * 128
                b[:, s * SW:s * SW + 128] = UR[:, g:g + 128]
                b[:, s * SW + 128:(s + 1) * SW] = VC[:, lo:lo + W]
            b[:, OFF_UO:OFF_UO + OVF] = uo
            b[:, OFF_VO:OFF_VO + OVW] = VC[:, c * OVW:(c + 1) * OVW]
            in_maps[c][f"b{d}"] = b
    return plans, in_maps


def _decode(plans, res, pI, pJ):
    out_idx = {}
    for d, pl in plans.items():
        n = pl["n"]
        los, pr, pc = pl["los"], pl["pr"], pl["pc"]
        idx_sorted = np.empty(n, np.int64)
        for c in range(NCORES):
            qi = res[c][f"qi{d}"].reshape(128, 8 * (STRIPES + OVB))
            for s in range(STRIPES):
                g = slice(c * RPC + s * 128, c * RPC + (s + 1) * 128)
                idx_sorted[g] = los[c * STRIPES + s] + qi[:, 8 * s].astype(np.int64)
        ovf = pl["ovf_used"]
        if len(ovf):
            vals = np.stack([res[c][f"qf{d}"].reshape(128, 8 * OVB)[:, ::8]
                             .T.reshape(-1).astype(np.float16) for c in range(NCORES)])
            idxs = np.stack([res[c][f"qi{d}"].reshape(128, 8 * (STRIPES + OVB))
                             [:, 8 * STRIPES::8].T.reshape(-1).astype(np.int64)
                             for c in range(NCORES)])
            k = np.arange(len(ovf))
            v = vals[:, k]
            best = v.max(axis=0)
            first_core = np.argmax(v == best[None, :], axis=0)
            idx_sorted[ovf] = first_core * OVW + idxs[first_core, k]
        # rare safety net: rows beyond OVF capacity, exact on host
        rows_pts = pI if d == 1 else pJ
        cols_pts = pJ if d == 1 else pI
        for r in pl["ovf_extra"]:
            p = rows_pts[pr[r]]
            d2 = ((cols_pts[pc].astype(np.float64) - p[None, :]) ** 2).sum(-1)
            idx_sorted[r] = int(np.argmin(d2))
        out = np.empty(n, np.int64)
        out[pr] = pc[idx_sorted]
        out_idx[d] = out
    return out_idx[1], out_idx[2]


def kernel(pointsI, pointsJ):
    pI = np.asarray(pointsI, dtype=np.float32)
    pJ = np.asarray(pointsJ, dtype=np.float32)

    if "nc" not in _prog_cache:
        _prog_cache["nc"] = _build_program()
    nc = _prog_cache["nc"]

    plans, in_maps = _prep(pI, pJ)
    res = run_bass_kernel_spmd(nc, in_maps, list(range(NCORES))).results
    idx1, idx2 = _decode(plans, res, pI, pJ)

    err_i = np.mean(np.abs(pI.astype(np.float64) - pJ[idx1].astype(np.float64)))
    err_j = np.mean(np.abs(pJ.astype(np.float64) - pI[idx2].astype(np.float64)))
    return np.array(err_i / N + err_j / M, dtype=np.float32)
